# revision 5
# baseline (speedup 1.0000x reference)
"""Trainium2 Bass kernel for nn_MultiHeadAttention_83863531421896.

Full-input contract: kernel(**inputs) takes the unsharded tensors and
returns the full (2, 2048, 1024) output. Internally the 16 heads are
sharded 2-per-core across 8 NeuronCores (tensor parallel); each core
computes its heads' attention plus its slice of the output projection,
and the 8 partial projections are reduced on the host.

Device dataflow per core (heads h0, h1):
  per batch b:
    qkvT = W_qkv_slice @ x^T  (fp32r, transposed layout [q_h0;q_h1],
           [k_h0;k_h1] stacked 64+64 partitions, vT), V^T -> V via PE
           transposes, V packed as [V | ones] blocks
    attention: per (q-chunk, kk-pair): S^T = K Q^T on dual 64-row PE
           tiles (both heads concurrently), exp((1/8) S^T) on ScalarE
           from PSUM, A^T V via [V|ones] stationary operand ->
           attention output + softmax denominators in one accumulation;
           normalization deferred to eviction (approx reciprocal)
    out-proj for the previous batch interleaves with the next batch's
           qkv so its DMA hides under attention
"""

import sys

if "/opt/trn_rl_repo" not in sys.path:
    sys.path.insert(0, "/opt/trn_rl_repo")

import numpy as np

B = 2
S = 2048
D = 1024
H = 16
HD = 64
N_CORES = 8
HEADS_PER_CORE = H // N_CORES  # 2
M = B * S                      # 4096 tokens
N_MCHUNK_B = S // 512          # 4 m-chunks of 512 tokens per batch
N_KTILE = D // 128             # 8 contraction tiles for qkv
N_QCHUNK = S // 512            # 4 q-chunks per batch
N_KKTILE = S // 128            # 16 key tiles per batch
SCALE = 1.0 / np.sqrt(HD)

_CACHE = {}


def _build_module():
    import concourse.bass as bass
    import concourse.tile as tile
    from concourse import bacc, mybir

    f32 = mybir.dt.float32
    f32r = mybir.dt.float32r
    bf16 = mybir.dt.bfloat16
    Exp = mybir.ActivationFunctionType.Exp
    Copy = mybir.ActivationFunctionType.Copy
    Ident = mybir.ActivationFunctionType.Identity

    nc = bacc.Bacc("TRN2", target_bir_lowering=False, debug=False,
                   num_devices=N_CORES)

    xt_ap = nc.dram_tensor("xt", [D, M], f32r, kind="ExternalInput").ap()
    wqa_ap = nc.dram_tensor("wqa", [D, 128], f32r, kind="ExternalInput").ap()
    wqb_ap = nc.dram_tensor("wqb", [D, 128], f32r, kind="ExternalInput").ap()
    wv_ap = nc.dram_tensor("wv", [D, 128], f32r, kind="ExternalInput").ap()
    wo_ap = nc.dram_tensor("wo", [128, D], f32r, kind="ExternalInput").ap()
    ba_ap = nc.dram_tensor("ba", [128, 1], f32, kind="ExternalInput").ap()
    bb_ap = nc.dram_tensor("bb", [128, 1], f32, kind="ExternalInput").ap()
    bv_ap = nc.dram_tensor("bv", [128, 1], f32, kind="ExternalInput").ap()
    ones_ap = nc.dram_tensor("ones", [128, 64], f32r, kind="ExternalInput").ap()
    ident_ap = nc.dram_tensor("ident", [128, 128], f32r, kind="ExternalInput").ap()
    out_ap = nc.dram_tensor("partial", [D, M], f32, kind="ExternalOutput").ap()
    sums_dram = nc.dram_tensor(
        "sums_scratch", [B * N_QCHUNK * HEADS_PER_CORE, 512], f32).ap()

    with tile.TileContext(nc) as tc:
        with tc.tile_pool(name="persist", bufs=1) as persist, \
             tc.tile_pool(name="const", bufs=1) as const, \
             tc.tile_pool(name="xpool", bufs=4) as xpool, \
             tc.tile_pool(name="vt_pool", bufs=2) as vt_pool, \
             tc.tile_pool(name="ps8", bufs=1, space="PSUM") as ps8, \
             tc.tile_pool(name="epool", bufs=2) as epool, \
             tc.tile_pool(name="stage", bufs=2) as stage, \
             tc.tile_pool(name="fin", bufs=4) as fin:
            qka_sb = persist.tile([128, M], bf16, tag="qka")
            qkb_sb = persist.tile([128, M], bf16, tag="qkb")
            v_sb = persist.tile([128, B, N_KKTILE, HEADS_PER_CORE, 65], f32r,
                                tag="vsb")
            outt_sb = persist.tile([128, M], f32r, tag="outt")

            wo_sb = const.tile([128, D], f32r, tag="wo")
            nc.scalar.dma_start(wo_sb[:], wo_ap[:])
            ident_sb = const.tile([128, 128], f32r, tag="ident")
            nc.scalar.dma_start(ident_sb[:], ident_ap[:])
            ba_sb = const.tile([128, 1], f32, tag="ba")
            nc.scalar.dma_start(ba_sb[:], ba_ap[:])
            bb_sb = const.tile([128, 1], f32, tag="bb")
            nc.scalar.dma_start(bb_sb[:], bb_ap[:])
            bv_sb = const.tile([128, 1], f32, tag="bv")
            nc.scalar.dma_start(bv_sb[:], bv_ap[:])
            wq_sb = const.tile([128, 3, N_KTILE, 128], f32r, tag="wq")
            for ki in range(N_KTILE):
                eng = (nc.gpsimd, nc.scalar, nc.gpsimd)[ki % 3]
                eng.dma_start(wq_sb[:, 0, ki], wqa_ap[ki * 128:(ki + 1) * 128, :])
                eng.dma_start(wq_sb[:, 1, ki], wqb_ap[ki * 128:(ki + 1) * 128, :])
                eng.dma_start(wq_sb[:, 2, ki], wv_ap[ki * 128:(ki + 1) * 128, :])
            nc.gpsimd.dma_start(
                v_sb[:, :, :, :, 64:65],
                ones_ap[:, 0:B * N_KKTILE * HEADS_PER_CORE].rearrange(
                    "p (b t h) -> p b t h", b=B, t=N_KKTILE)[:, :, :, :, None])

            def qkv_phase(b2):
                vt_sb = vt_pool.tile([128, S], f32r, tag="vt", name=f"vt{b2}")
                xss = []
                for mc in range(N_MCHUNK_B):
                    mi = b2 * N_MCHUNK_B + mc
                    xs = xpool.tile([128, N_KTILE, 512], f32r, tag="xs",
                                    name=f"xs{mi}")
                    for ki in range(N_KTILE):
                        eng = (nc.sync, nc.gpsimd, nc.sync, nc.scalar,
                               nc.sync, nc.gpsimd, nc.sync, nc.scalar)[ki]
                        eng.dma_start(
                            xs[:, ki],
                            xt_ap[ki * 128:(ki + 1) * 128, mi * 512:(mi + 1) * 512])
                    xss.append(xs)
                # ki-inner-most over m-chunks: one weight load feeds 4 matmuls
                for ei, (bias, dest) in enumerate(
                        [(ba_sb, qka_sb), (bb_sb, qkb_sb), (bv_sb, vt_sb)]):
                    pss = [ps8.tile([128, 512], f32, tag=f"av{mc // 2}{mc % 2}",
                                    name=f"qkvps{mc}") for mc in range(N_MCHUNK_B)]
                    for ki in range(N_KTILE):
                        for mc in range(N_MCHUNK_B):
                            nc.tensor.matmul(pss[mc][:], wq_sb[:, ei, ki],
                                             xss[mc][:, ki],
                                             start=(ki == 0), stop=(ki == N_KTILE - 1))
                    for mc in range(N_MCHUNK_B):
                        col = (b2 * N_MCHUNK_B + mc) if ei < 2 else mc
                        nc.vector.tensor_scalar_add(
                            dest[:, col * 512:(col + 1) * 512], pss[mc][:], bias[:])
                for kt in range(N_KKTILE):
                    tp = ps8.tile([128, 128], f32r, tag=f"av{kt % 2}1", name="tp")
                    nc.tensor.transpose(tp[:], vt_sb[:, kt * 128:(kt + 1) * 128],
                                        ident_sb[:])
                    for h in range(HEADS_PER_CORE):
                        nc.vector.tensor_copy(v_sb[:, b2, kt, h, 0:64],
                                              tp[:, h * 64:(h + 1) * 64])

            def attn_phase(b2):
                for qi in range(N_QCHUNK):
                    qcol = b2 * S + qi * 512
                    avp = [[ps8.tile([128, 512], f32, tag=f"av{h}{par}",
                                     name=f"av{h}{par}")
                            for par in range(2)] for h in range(HEADS_PER_CORE)]
                    def emit_av(kt, es_kt):
                        first = (kt == 0)
                        last = (kt == N_KKTILE - 1)
                        for h in range(HEADS_PER_CORE):
                            nc.tensor.matmul(
                                avp[h][0][0:65, :],
                                v_sb[:, b2, kt, h, :],
                                es_kt[h][:, :],
                                start=first, stop=last)

                    pending = None
                    for kt in range(N_KKTILE):
                        kkcol = b2 * S + kt * 128
                        scs = []
                        for h in range(HEADS_PER_CORE):
                            # scores: T0/T8 alternation (h0 parts 0:64, h1
                            # parts 64:128); bufs=2 so the next tile's scores
                            # don't wait on this tile's exp
                            sc = ps8.tile([128, 512], f32, tag=f"sc{h}",
                                          bufs=2, name=f"sc{h}")
                            nc.tensor.matmul(
                                sc[:],
                                qkb_sb[h * 64:(h + 1) * 64, kkcol:kkcol + 128],
                                qka_sb[h * 64:(h + 1) * 64, qcol:qcol + 512],
                                start=True, stop=True)
                            scs.append(sc)
                        es = []
                        for h in range(HEADS_PER_CORE):
                            e = epool.tile([128, 512], f32r, tag=f"e{h}",
                                           bufs=4, name=f"e{h}")
                            nc.scalar.activation(e[:], scs[h][:], Exp, scale=SCALE)
                            es.append(e)
                        if pending is not None:
                            emit_av(*pending)
                        pending = (kt, es)
                    emit_av(*pending)
                    for h in range(HEADS_PER_CORE):
                        st = stage.tile([128, 512], f32, tag="st", name="st")
                        nc.vector.tensor_copy(st[0:65, :], avp[h][0][0:65, :])
                        sidx = (b2 * N_QCHUNK + qi) * HEADS_PER_CORE + h
                        nc.gpsimd.dma_start(sums_dram[sidx:sidx + 1, :],
                                            st[64:65, :])
                        rb = stage.tile([128, 512], f32, tag="rb", name="rb")
                        nc.gpsimd.dma_start(
                            rb[0:64, :],
                            sums_dram[sidx:sidx + 1, :]
                            .partition_broadcast(64).squeeze(1))
                        rb2 = stage.tile([128, 512], f32, tag="rb2", name="rb2")
                        nc.vector.reciprocal_approx_fast(rb2[0:64, :], rb[0:64, :])
                        if h == 0:
                            nc.vector.tensor_mul(outt_sb[0:64, qcol:qcol + 512],
                                                 st[0:64, :], rb2[0:64, :])
                        else:
                            tm = stage.tile([128, 512], f32r, tag="tm", name="tm")
                            nc.vector.tensor_mul(tm[0:64, :], st[0:64, :],
                                                 rb2[0:64, :])
                            nc.gpsimd.dma_start(outt_sb[64:128, qcol:qcol + 512],
                                                tm[0:64, :])

            def outproj_chunk(b2, mc):
                # finalT[e, m-chunk] = wo_sb[:, e-tile].T @ outT[:, m-chunk]
                for et in range(D // 128):
                    mrow = b2 * S + mc * 512
                    fp = ps8.tile([128, 512], f32, tag=f"av{et % 2}0",
                                  name="fp")
                    nc.tensor.matmul(fp[:],
                                     wo_sb[:, et * 128:(et + 1) * 128],
                                     outt_sb[:, mrow:mrow + 512],
                                     start=True, stop=True)
                    fo = fin.tile([128, 512], f32, tag="fo", name="fo")
                    nc.vector.tensor_copy(fo[:], fp[:])
                    nc.sync.dma_start(
                        out_ap[et * 128:(et + 1) * 128, mrow:mrow + 512],
                        fo[:])

            qkv_phase(0)
            attn_phase(0)
            qkv_phase(1)
            for mc in range(N_MCHUNK_B):
                outproj_chunk(0, mc)
            tc.no_sync_barrier()
            attn_phase(1)
            tc.no_sync_barrier()
            for mc in range(N_MCHUNK_B):
                outproj_chunk(1, mc)
    nc.compile()
    return nc


def _shard_inputs(x, w_qkv, b_qkv, w_out):
    xt = np.ascontiguousarray(x.reshape(M, D).T)  # (1024, 4096)
    ones = np.ones((128, 64), dtype=np.float32)
    ident = np.eye(128, dtype=np.float32)
    in_maps = []
    for c in range(N_CORES):
        h0 = HEADS_PER_CORE * c
        rows_q, rows_k, rows_v, dcols = [], [], [], []
        for h in (h0, h0 + 1):
            rows_q += list(range(h * 192, h * 192 + 64))
            rows_k += list(range(h * 192 + 64, h * 192 + 128))
            rows_v += list(range(h * 192 + 128, h * 192 + 192))
            dcols += list(range(h * 64, (h + 1) * 64))
        in_maps.append({
            "xt": xt,
            "wqa": np.ascontiguousarray(w_qkv[rows_q, :].T),
            "wqb": np.ascontiguousarray(w_qkv[rows_k, :].T),
            "wv": np.ascontiguousarray(w_qkv[rows_v, :].T),
            "wo": np.ascontiguousarray(w_out[:, dcols].T),
            "ba": np.ascontiguousarray(b_qkv[rows_q].reshape(128, 1)),
            "bb": np.ascontiguousarray(b_qkv[rows_k].reshape(128, 1)),
            "bv": np.ascontiguousarray(b_qkv[rows_v].reshape(128, 1)),
            "ones": ones,
            "ident": ident,
        })
    return in_maps


def kernel(x, w_qkv, b_qkv, w_out, b_out, _trace=False):
    from concourse.bass_utils import run_bass_kernel_spmd

    x = np.asarray(x, dtype=np.float32)
    w_qkv = np.asarray(w_qkv, dtype=np.float32)
    b_qkv = np.asarray(b_qkv, dtype=np.float32)
    w_out = np.asarray(w_out, dtype=np.float32)
    b_out = np.asarray(b_out, dtype=np.float32)

    if "nc" not in _CACHE:
        _CACHE["nc"] = _build_module()
    nc = _CACHE["nc"]

    in_maps = _shard_inputs(x, w_qkv, b_qkv, w_out)
    res = run_bass_kernel_spmd(nc, in_maps, list(range(N_CORES)), trace=_trace)
    acc = np.zeros((D, M), dtype=np.float64)
    for c in range(N_CORES):
        acc += res.results[c]["partial"]
    acc = acc.T + b_out
    out = acc.astype(np.float32).reshape(B, S, D)
    if _trace:
        _CACHE["last_exec_time_ns"] = res.exec_time_ns
        _CACHE["last_res"] = res
    return out



# revision 13
# speedup vs baseline: 1.5644x; 1.5644x over previous
"""Trainium2 Bass kernel for nn_MultiHeadAttention_83863531421896.

Full-input contract: kernel(**inputs) takes the unsharded tensors and
returns the full (2, 2048, 1024) output. Internally the 16 heads are
sharded 2-per-core across 8 NeuronCores (tensor parallel); each core
computes its heads' attention plus its slice of the output projection,
and the 8 partial projections are reduced on the host.

Device dataflow per core (heads h0, h1), all matmul operands bf16:
  per batch b:
    qkvT = W_qkv_slice @ x^T, V^T -> V via PE transposes,
           V packed as [V | ones] blocks
    attention: per (q-chunk, kk-pair): both heads' scores as a
           64-row-quadrant PE pair into one 2-bank PSUM tile,
           ONE [128,1024] exp on Scalar (scale folded), A^T V via
           [V|ones] stationary par-split quadrant pairs ->
           attention output + softmax denominators;
           normalization deferred to eviction (approx reciprocal)
    out-proj for the previous batch interleaves with the next batch's
           qkv so its DMA hides under attention
"""

import sys

if "/opt/trn_rl_repo" not in sys.path:
    sys.path.insert(0, "/opt/trn_rl_repo")

import numpy as np

B = 2
S = 2048
D = 1024
H = 16
HD = 64
N_CORES = 8
HEADS_PER_CORE = H // N_CORES  # 2
M = B * S                      # 4096 tokens
N_MCHUNK_B = S // 512          # 4 m-chunks of 512 tokens per batch
N_KTILE = D // 128             # 8 contraction tiles for qkv
N_QCHUNK = S // 512            # 4 q-chunks per batch
N_KKTILE = S // 128            # 16 key tiles per batch
SCALE = 1.0 / np.sqrt(HD)

_CACHE = {}


def _build_module():
    import concourse.bass as bass
    import concourse.tile as tile
    from concourse import bacc, mybir

    f32 = mybir.dt.float32
    bf16 = mybir.dt.bfloat16
    Exp = mybir.ActivationFunctionType.Exp

    nc = bacc.Bacc("TRN2", target_bir_lowering=False, debug=False,
                   num_devices=N_CORES)

    xt_ap = nc.dram_tensor("xt", [D, M], bf16, kind="ExternalInput").ap()
    wqa_ap = nc.dram_tensor("wqa", [D, 128], bf16, kind="ExternalInput").ap()
    wqb_ap = nc.dram_tensor("wqb", [D, 128], bf16, kind="ExternalInput").ap()
    wv_ap = nc.dram_tensor("wv", [D, 128], bf16, kind="ExternalInput").ap()
    wo_ap = nc.dram_tensor("wo", [128, D], bf16, kind="ExternalInput").ap()
    ba_ap = nc.dram_tensor("ba", [128, 1], f32, kind="ExternalInput").ap()
    bb_ap = nc.dram_tensor("bb", [128, 1], f32, kind="ExternalInput").ap()
    bv_ap = nc.dram_tensor("bv", [128, 1], f32, kind="ExternalInput").ap()
    ones_ap = nc.dram_tensor("ones", [128, 64], bf16, kind="ExternalInput").ap()
    ident_ap = nc.dram_tensor("ident", [128, 128], bf16,
                              kind="ExternalInput").ap()
    out_ap = nc.dram_tensor("partial", [D, M], bf16, kind="ExternalOutput").ap()
    sums_dram = nc.dram_tensor(
        "sums_scratch", [B * N_QCHUNK * HEADS_PER_CORE, 512], f32).ap()

    with tile.TileContext(nc) as tc:
        with tc.tile_pool(name="persist", bufs=1) as persist, \
             tc.tile_pool(name="const", bufs=1) as const, \
             tc.tile_pool(name="xpool", bufs=4) as xpool, \
             tc.tile_pool(name="vt_pool", bufs=2) as vt_pool, \
             tc.tile_pool(name="ps8", bufs=1, space="PSUM") as ps8, \
             tc.tile_pool(name="epool", bufs=2) as epool, \
             tc.tile_pool(name="stage", bufs=2) as stage, \
             tc.tile_pool(name="fin", bufs=4) as fin:
            qka_sb = persist.tile([128, M], bf16, tag="qka")
            qkb_sb = persist.tile([128, M], bf16, tag="qkb")
            v_sb = persist.tile([128, B, N_KKTILE, HEADS_PER_CORE, 65], bf16,
                                tag="vsb")
            outt_sb = persist.tile([128, M], bf16, tag="outt")

            wo_sb = const.tile([128, D], bf16, tag="wo")
            nc.sync.dma_start(wo_sb[:], wo_ap[:])
            ident_sb = const.tile([128, 128], bf16, tag="ident")
            nc.sync.dma_start(ident_sb[:], ident_ap[:])
            ba_sb = const.tile([128, 1], f32, tag="ba")
            nc.sync.dma_start(ba_sb[:], ba_ap[:])
            bb_sb = const.tile([128, 1], f32, tag="bb")
            nc.sync.dma_start(bb_sb[:], bb_ap[:])
            bv_sb = const.tile([128, 1], f32, tag="bv")
            nc.sync.dma_start(bv_sb[:], bv_ap[:])
            wq_sb = const.tile([128, 3, N_KTILE, 128], bf16, tag="wq")
            for ki in range(N_KTILE):
                eng = (nc.gpsimd, nc.sync, nc.gpsimd)[ki % 3]
                eng.dma_start(wq_sb[:, 0, ki], wqa_ap[ki * 128:(ki + 1) * 128, :])
                eng.dma_start(wq_sb[:, 1, ki], wqb_ap[ki * 128:(ki + 1) * 128, :])
                eng.dma_start(wq_sb[:, 2, ki], wv_ap[ki * 128:(ki + 1) * 128, :])
            nc.gpsimd.dma_start(
                v_sb[:, :, :, :, 64:65],
                ones_ap[:, 0:B * N_KKTILE * HEADS_PER_CORE].rearrange(
                    "p (b t h) -> p b t h", b=B, t=N_KKTILE)[:, :, :, :, None])

            def qkv_phase(b2):
                vt_sb = vt_pool.tile([128, S], bf16, tag="vt", name=f"vt{b2}")
                xss = [xpool.tile([128, N_KTILE, 512], bf16, tag="xs",
                                  name=f"xs{b2 * N_MCHUNK_B + mc}")
                       for mc in range(N_MCHUNK_B)]
                # ki-major so the ki=0 column of all m-chunks lands first
                # and the matmul stream can start almost immediately
                for ki in range(N_KTILE):
                    for mc in range(N_MCHUNK_B):
                        mi = b2 * N_MCHUNK_B + mc
                        eng = (nc.sync, nc.gpsimd)[(ki * N_MCHUNK_B + mc) % 2]
                        eng.dma_start(
                            xss[mc][:, ki],
                            xt_ap[ki * 128:(ki + 1) * 128,
                                  mi * 512:(mi + 1) * 512])
                # ki-inner-most over m-chunks: one weight load feeds 4 matmuls
                for ei, (bias, dest) in enumerate(
                        [(ba_sb, qka_sb), (bb_sb, qkb_sb), (bv_sb, vt_sb)]):
                    pss = [ps8.tile([128, 512], f32, tag=f"av{mc // 2}{mc % 2}",
                                    name=f"qkvps{mc}") for mc in range(N_MCHUNK_B)]
                    for ki in range(N_KTILE):
                        for mc in range(N_MCHUNK_B):
                            nc.tensor.matmul(pss[mc][:], wq_sb[:, ei, ki],
                                             xss[mc][:, ki],
                                             start=(ki == 0), stop=(ki == N_KTILE - 1))
                    for mc in range(N_MCHUNK_B):
                        col = (b2 * N_MCHUNK_B + mc) if ei < 2 else mc
                        nc.vector.tensor_scalar_add(
                            dest[:, col * 512:(col + 1) * 512], pss[mc][:], bias[:])
                for kt in range(N_KKTILE):
                    tp = ps8.tile([128, 128], bf16, tag=f"av{kt % 2}1",
                                  name="tp")
                    nc.tensor.transpose(tp[:], vt_sb[:, kt * 128:(kt + 1) * 128],
                                        ident_sb[:])
                    for h in range(HEADS_PER_CORE):
                        nc.vector.tensor_copy(v_sb[:, b2, kt, h, 0:64],
                                              tp[:, h * 64:(h + 1) * 64])

            def attn_phase(b2):
                for qi in range(N_QCHUNK):
                    qcol = b2 * S + qi * 512
                    avp = [[ps8.tile([128, 512], f32, tag=f"av{h}{par}",
                                     name=f"av{h}{par}")
                            for par in range(2)] for h in range(HEADS_PER_CORE)]

                    def emit_av(kt, es_kt):
                        first = (kt == 0)
                        last = (kt == N_KKTILE - 1)
                        for h in range(HEADS_PER_CORE):
                            for par in range(2):
                                nc.tensor.matmul(
                                    avp[h][par][0:65, :],
                                    v_sb[par * 64:par * 64 + 64, b2, kt, h, :],
                                    es_kt[par * 64:par * 64 + 64, h, :],
                                    start=first, stop=last)

                    pending = None
                    for kt in range(N_KKTILE):
                        kkcol = b2 * S + kt * 128
                        # both heads' scores into one 2-bank PSUM tile so a
                        # single [128,1024] exp serves the pair; bufs=2 so the
                        # next tile's scores don't wait on this tile's exp
                        sc = ps8.tile([128, 2, 512], f32, tag="sc", bufs=2,
                                      name="sc")
                        for h in range(HEADS_PER_CORE):
                            nc.tensor.matmul(
                                sc[:, h, :],
                                qkb_sb[h * 64:(h + 1) * 64, kkcol:kkcol + 128],
                                qka_sb[h * 64:(h + 1) * 64, qcol:qcol + 512],
                                start=True, stop=True)
                        e = epool.tile([128, 2, 512], bf16, tag="e",
                                       bufs=4, name="e")
                        nc.scalar.activation(e[:], sc[:], Exp, scale=SCALE)
                        if pending is not None:
                            emit_av(*pending)
                        pending = (kt, e)
                    emit_av(*pending)

                    sidx = (b2 * N_QCHUNK + qi) * HEADS_PER_CORE
                    for h in range(HEADS_PER_CORE):
                        st1 = stage.tile([128, 512], f32, tag=f"st1{h}",
                                         name="st1")
                        nc.vector.tensor_copy(st1[0:65, :], avp[h][1][0:65, :])
                        st = stage.tile([128, 512], f32, tag=f"st{h}",
                                        name="st")
                        nc.vector.tensor_add(st[0:65, :], avp[h][0][0:65, :],
                                             st1[0:65, :])
                        nc.gpsimd.dma_start(sums_dram[sidx + h:sidx + h + 1, :],
                                            st[64:65, :])
                        rb = stage.tile([128, 512], f32, tag="rb", name="rb")
                        nc.gpsimd.dma_start(
                            rb[0:64, :],
                            sums_dram[sidx + h:sidx + h + 1, :]
                            .partition_broadcast(64).squeeze(1))
                        rb2 = stage.tile([128, 512], f32, tag="rb2", name="rb2")
                        nc.vector.reciprocal_approx_fast(rb2[0:64, :],
                                                         rb[0:64, :])
                        if h == 0:
                            nc.vector.tensor_mul(
                                outt_sb[0:64, qcol:qcol + 512],
                                st[0:64, :], rb2[0:64, :])
                        else:
                            tm = stage.tile([128, 512], bf16, tag="tm",
                                            name="tm")
                            nc.vector.tensor_mul(tm[0:64, :], st[0:64, :],
                                                 rb2[0:64, :])
                            nc.gpsimd.dma_start(
                                outt_sb[64:128, qcol:qcol + 512], tm[0:64, :])

            def outproj_chunk(b2, mc):
                # finalT[e, m-chunk] = wo_sb[:, e-tile].T @ outT[:, m-chunk]
                for et in range(D // 128):
                    mrow = b2 * S + mc * 512
                    fp = ps8.tile([128, 512], f32, tag=f"av{et % 2}0",
                                  name="fp")
                    nc.tensor.matmul(fp[:],
                                     wo_sb[:, et * 128:(et + 1) * 128],
                                     outt_sb[:, mrow:mrow + 512],
                                     start=True, stop=True)
                    fo = fin.tile([128, 512], bf16, tag="fo", name="fo")
                    nc.vector.tensor_copy(fo[:], fp[:])
                    nc.sync.dma_start(
                        out_ap[et * 128:(et + 1) * 128, mrow:mrow + 512],
                        fo[:])

            qkv_phase(0)
            attn_phase(0)
            qkv_phase(1)
            for mc in range(N_MCHUNK_B):
                outproj_chunk(0, mc)
            tc.no_sync_barrier()
            attn_phase(1)
            tc.no_sync_barrier()
            for mc in range(N_MCHUNK_B):
                outproj_chunk(1, mc)
    nc.compile()
    return nc


def _shard_inputs(x, w_qkv, b_qkv, w_out):
    import ml_dtypes

    bf = ml_dtypes.bfloat16
    xt = np.ascontiguousarray(x.reshape(M, D).T).astype(bf)  # (1024, 4096)
    ones = np.ones((128, 64), dtype=bf)
    ident = np.eye(128, dtype=bf)
    in_maps = []
    for c in range(N_CORES):
        h0 = HEADS_PER_CORE * c
        rows_q, rows_k, rows_v, dcols = [], [], [], []
        for h in (h0, h0 + 1):
            rows_q += list(range(h * 192, h * 192 + 64))
            rows_k += list(range(h * 192 + 64, h * 192 + 128))
            rows_v += list(range(h * 192 + 128, h * 192 + 192))
            dcols += list(range(h * 64, (h + 1) * 64))
        in_maps.append({
            "xt": xt,
            "wqa": np.ascontiguousarray(w_qkv[rows_q, :].T).astype(bf),
            "wqb": np.ascontiguousarray(w_qkv[rows_k, :].T).astype(bf),
            "wv": np.ascontiguousarray(w_qkv[rows_v, :].T).astype(bf),
            "wo": np.ascontiguousarray(w_out[:, dcols].T).astype(bf),
            "ba": np.ascontiguousarray(b_qkv[rows_q].reshape(128, 1)),
            "bb": np.ascontiguousarray(b_qkv[rows_k].reshape(128, 1)),
            "bv": np.ascontiguousarray(b_qkv[rows_v].reshape(128, 1)),
            "ones": ones,
            "ident": ident,
        })
    return in_maps


def kernel(x, w_qkv, b_qkv, w_out, b_out, _trace=False):
    from concourse.bass_utils import run_bass_kernel_spmd

    x = np.asarray(x, dtype=np.float32)
    w_qkv = np.asarray(w_qkv, dtype=np.float32)
    b_qkv = np.asarray(b_qkv, dtype=np.float32)
    w_out = np.asarray(w_out, dtype=np.float32)
    b_out = np.asarray(b_out, dtype=np.float32)

    if "nc" not in _CACHE:
        _CACHE["nc"] = _build_module()
    nc = _CACHE["nc"]

    in_maps = _shard_inputs(x, w_qkv, b_qkv, w_out)
    res = run_bass_kernel_spmd(nc, in_maps, list(range(N_CORES)), trace=_trace)
    acc = np.zeros((D, M), dtype=np.float64)
    for c in range(N_CORES):
        acc += res.results[c]["partial"].astype(np.float32)
    acc = acc.T + b_out
    out = acc.astype(np.float32).reshape(B, S, D)
    if _trace:
        _CACHE["last_exec_time_ns"] = res.exec_time_ns
        _CACHE["last_res"] = res
    return out
